# revision 22
# baseline (speedup 1.0000x reference)
"""Bidirectional Mamba2 layer on 8 NeuronCores (v2).

Sharding: 8 cores = 4 batch elements x 2 directions (fw/bw). Each core runs
one full Mamba2 layer on one sequence; host flips bw sequences, adds fw+bw,
applies the padding mask.

Per-core kernel (L=2048, chunked SSD scan, T=128), redesigned from the
baseline for engine balance:
  1. in_proj channel-major matmuls; depthwise conv interleaved per channel
     tile so DVE conv overlaps PE in_proj.
  2. decay matrices via cumsum S = UINC@ld; P[j,(h,i)] = S_h[i] + (ln dt -
     S)_h[j] + mask built with 3 matmuls per 4-head block (rank-1 bcast,
     K=4 indicator, K=128 mask); one exp per block; m = gt_bcast * e.
  3. state: one broadcast-scaled xdtw; 2 N=512 matmuls per chunk.
  4. inter-Y computed time-major (C_cm^T @ S_prev), scaled by w16 broadcast,
     transpose-accumulated into channel-major y PSUM via identity matmuls.
  5. gating + RMSNorm per chunk during the scan (norm_w folded into w_out
     host-side); out_proj per 512-column block.
"""

import numpy as np

D_MODEL = 512
D_STATE = 128
NH = 16
HD = 64
D_INNER = 1024
D_XBC = 1280
D_IN = 2320
L = 2048
T = 128
NCH = L // T
B_SZ = 4
EPS = 1e-5
NEG_INF = -1e30

_CACHE = {}


def _patch_drain(tile, mybir, ScopedClock):
    # workaround: this walrus build rejects >2 sem waits per instruction;
    # spread the TileContext exit-drain waits across nop instructions.
    def _drain_and_barrier(self, tick_clock, wait_clock):
        nc_ = self.nc
        probe = nc_.sync.nop()
        wait_clock.add_sem_waits(
            probe.ins, ScopedClock({None: tick_clock.global_clock})
        )
        waits = list(probe.ins.sync_info.on_wait or [])
        if probe.ins.sync_info is not None:
            probe.ins.sync_info.on_wait = waits[:1]
            rest = waits[1:]
        else:
            rest = []
        for w in rest:
            n = nc_.sync.nop()
            if n.ins.sync_info is None:
                n.ins.sync_info = mybir.SyncInfo(on_wait=[w], on_update=[])
            else:
                n.ins.sync_info.on_wait = [w]
        nc_.sync.drain()
        nc_.all_engine_barrier()
        assert self.sems is not None
        popped = nc_._tile_sem_poison_stack.pop()
        assert popped is self._sem_poison
        nc_.clear_and_free_semaphores(list(self.sems.allocated().values()))
        nc_.all_engine_barrier()

    tile.TileContext._drain_and_barrier = _drain_and_barrier


def _build_program(dbg=False):
    import concourse.bass as bass
    import concourse.mybir as mybir
    import concourse.tile as tile
    from concourse.vector_clock import ScopedClock

    _patch_drain(tile, mybir, ScopedClock)

    f32 = mybir.dt.float32
    bf16 = mybir.dt.bfloat16
    AF = mybir.ActivationFunctionType
    OP = mybir.AluOpType

    nc = bass.Bass("TRN2", target_bir_lowering=False, debug=False)

    # ---------------- DRAM I/O ----------------
    xT_d = nc.dram_tensor("xT", [D_MODEL, L], bf16, kind="ExternalInput")
    w_in_d = nc.dram_tensor("w_in", [D_MODEL, D_IN], bf16, kind="ExternalInput")
    w_out_d = nc.dram_tensor("w_out", [D_INNER, D_MODEL], bf16, kind="ExternalInput")
    convw_d = nc.dram_tensor("convw", [128, 10, 4], f32, kind="ExternalInput")
    convb_d = nc.dram_tensor("convb", [128, 10], f32, kind="ExternalInput")
    dtb_d = nc.dram_tensor("dtb", [16, 1], f32, kind="ExternalInput")
    nae_d = nc.dram_tensor("nae", [16, 1], f32, kind="ExternalInput")  # -exp(A_log)
    dcol_d = nc.dram_tensor("dcol", [128, 8], f32, kind="ExternalInput")
    alow_d = nc.dram_tensor("alow", [128, 128], bf16, kind="ExternalInput")
    uinc_d = nc.dram_tensor("uinc", [128, 128], bf16, kind="ExternalInput")
    idnb_d = nc.dram_tensor("idnb", [128, 128], bf16, kind="ExternalInput")
    idnf_d = nc.dram_tensor("idnf", [128, 128], f32, kind="ExternalInput")
    ones_d = nc.dram_tensor("ones", [128, 1], bf16, kind="ExternalInput")
    onesrf_d = nc.dram_tensor("onesrf", [1, 128], f32, kind="ExternalInput")
    onesrb_d = nc.dram_tensor("onesrb", [1, 128], bf16, kind="ExternalInput")
    minf4_d = nc.dram_tensor("minf4", [128, 512], bf16, kind="ExternalInput")
    ind4_d = nc.dram_tensor("ind4", [4, 512], f32, kind="ExternalInput")
    yT_d = nc.dram_tensor("yT", [D_MODEL, L], f32, kind="ExternalOutput")
    if dbg:
        xbcc_o = nc.dram_tensor("xbcc_o", [128, 10, L], bf16, kind="ExternalOutput")
        sz_o = nc.dram_tensor("sz_o", [128, 8, L], bf16, kind="ExternalOutput")
        y_o = nc.dram_tensor("y_o", [128, 8, L], bf16, kind="ExternalOutput")
        gn_o = nc.dram_tensor("gn_o", [128, 8, L], bf16, kind="ExternalOutput")
        s_o = nc.dram_tensor("s_o", [128, NCH, NH * HD], bf16, kind="ExternalOutput")

    with tile.TileContext(nc) as tc:
        with (
            tc.tile_pool(name="const", bufs=1) as cpool,
            tc.tile_pool(name="dram", bufs=1, space="DRAM") as dpool,
            tc.tile_pool(name="mid", bufs=1) as mid,
            tc.tile_pool(name="psSm", bufs=2, space="PSUM") as psSm,
        ):
            # ---------------- constants ----------------
            ALOW = cpool.tile([128, 128], bf16, tag="alow")
            nc.sync.dma_start(ALOW[:], alow_d.ap())
            UINC = cpool.tile([128, 128], bf16, tag="uinc")
            nc.sync.dma_start(UINC[:], uinc_d.ap())
            IDNB = cpool.tile([128, 128], bf16, tag="idnb")
            nc.sync.dma_start(IDNB[:], idnb_d.ap())
            IDNF = cpool.tile([128, 128], f32, tag="idnf")
            nc.sync.dma_start(IDNF[:], idnf_d.ap())
            ONEC = cpool.tile([128, 1], bf16, tag="ones")
            nc.sync.dma_start(ONEC[:], ones_d.ap())
            ONESRF = cpool.tile([1, 128], f32, tag="onesrf")
            nc.sync.dma_start(ONESRF[:], onesrf_d.ap())
            ONESRB = cpool.tile([1, 128], bf16, tag="onesrb")
            nc.sync.dma_start(ONESRB[:], onesrb_d.ap())
            MINF4 = cpool.tile([128, 512], bf16, tag="minf4")
            nc.sync.dma_start(MINF4[:], minf4_d.ap())
            IND4 = cpool.tile([4, 512], f32, tag="ind4")
            nc.sync.dma_start(IND4[:], ind4_d.ap())
            CONVW = cpool.tile([128, 10, 4], f32, tag="convw")
            nc.sync.dma_start(CONVW[:], convw_d.ap())
            CONVB = cpool.tile([128, 10], f32, tag="convb")
            nc.sync.dma_start(CONVB[:], convb_d.ap())
            DTB = cpool.tile([16, 1], f32, tag="dtb")
            nc.sync.dma_start(DTB[:], dtb_d.ap())
            NAE = cpool.tile([16, 1], f32, tag="nae")
            nc.sync.dma_start(NAE[:], nae_d.ap())
            DCOL = cpool.tile([128, 8], f32, tag="dcol")
            nc.sync.dma_start(DCOL[:], dcol_d.ap())
            EPSC = cpool.tile([128, 1], f32, tag="epsc")
            nc.vector.memset(EPSC[:], EPS)

            # ---------------- persistent tensors ----------------
            dtldT = mid.tile([128, NCH, 32], f32, tag="dtldT")  # 0:16 dt, 16:32 ld
            TRall = mid.tile([128, NCH, 32], f32, tag="TRall")  # 0:16 S, 16:32 lndt-S
            atot = mid.tile([16, 16], f32, tag="atot")          # [head, chunk]
            atotT = mid.tile([16, 16], f32, tag="atotT")
            atotF = mid.tile([1, 256], f32, tag="atotF")
            wdin_all = mid.tile([128, NCH, 32], f32, tag="wdin_all")
            dtw_all = mid.tile([128, NCH, 16], f32, tag="dtw_all")
            atb_all = mid.tile([128, NCH, 16], f32, tag="atb_all")
            s_sb = [mid.tile([128, NH, HD], bf16, tag=f"s_sb{i}", name=f"s_sb{i}")
                    for i in range(2)]
            xbc_c = mid.tile([128, 10, L], bf16, tag="xbc_c")
            sz = mid.tile([128, 8, L], bf16, tag="sz")
            wo = mid.tile([128, 8, D_MODEL], bf16, tag="wo")
            nc.sync.dma_start(
                wo[:], w_out_d.ap().rearrange("(ko p) m -> p ko m", p=128))

            rt_dram = dpool.tile([1152, L], bf16)

            # ============ PHASE 1: in_proj + conv + dt pipeline ============
            with (
                tc.tile_pool(name="pA", bufs=1) as pA,
                tc.tile_pool(name="pW", bufs=3) as pW,
                tc.tile_pool(name="pC", bufs=2) as pC,
                tc.tile_pool(name="ps1", bufs=4, space="PSUM") as ps1,
                tc.tile_pool(name="psT", bufs=2, space="PSUM") as psT,
            ):
                dtld = pA.tile([96, L], f32, tag="dtld")  # 0:16 dt, 32:48 scr, 64:80 ld
                xTs = pA.tile([128, 4, L], bf16, tag="xTs")
                xbc_pre = pA.tile([128, 10, L + 3], bf16, tag="xbc_pre")
                xTr = xT_d.ap().rearrange("(ko p) t -> p ko t", p=128)
                wir = w_in_d.ap().rearrange("(ko p) m -> p ko m", p=128)
                for k in range(4):
                    nc.sync.dma_start(xTs[:, k, :], xTr[:, k, :])
                nc.vector.memset(xbc_pre[:, :, 0:3], 0.0)

                def conv_tile(t):
                    acc = pC.tile([128, L], bf16, tag="conv_acc")
                    nc.vector.tensor_scalar_mul(
                        acc[:], xbc_pre[:, t, 0:L], CONVW[:, t, 0:1])
                    for k in range(1, 4):
                        nc.vector.scalar_tensor_tensor(
                            acc[:], xbc_pre[:, t, k:k + L],
                            CONVW[:, t, k:k + 1], acc[:],
                            op0=OP.mult, op1=OP.add,
                        )
                    nc.scalar.activation(
                        xbc_c[:, t, :], acc[:], AF.Silu, bias=CONVB[:, t:t + 1])
                    if t < 9:
                        nc.sync.dma_start(
                            rt_dram[t * 128:(t + 1) * 128, :], xbc_c[:, t, :])

                for m in [18] + list(range(8, 18)) + list(range(0, 8)):
                    mp = 128 if m < 18 else 16
                    wis = pW.tile([128, 4, 128], bf16, tag="wis")
                    for k in range(4):
                        nc.sync.dma_start(wis[:, k, 0:mp], wir[:, k, m * 128:m * 128 + mp])
                    for tb in range(4):
                        tsl = slice(tb * 512, (tb + 1) * 512)
                        ps = ps1.tile([128, 512], f32, tag="ps_inproj")
                        for k in range(4):
                            nc.tensor.matmul(
                                ps[:mp, :], wis[:, k, 0:mp], xTs[:, k, tsl],
                                start=(k == 0), stop=(k == 3),
                            )
                        if m < 8:
                            nc.scalar.activation(sz[:, m, tsl], ps[:, :], AF.Silu)
                        elif m < 18:
                            nc.scalar.copy(
                                xbc_pre[:, m - 8, 3 + tb * 512: 3 + (tb + 1) * 512],
                                ps[:, :])
                        else:
                            nc.scalar.copy(dtld[32:48, tsl], ps[:16, :])

                    if m == 18:
                        # dt = softplus(pre + dtb); ld = -exp(A_log) * dt
                        nc.scalar.activation(dtld[32:48, :], dtld[32:48, :], AF.Exp,
                                             bias=DTB[:, 0:1])
                        nc.scalar.activation(dtld[0:16, :], dtld[32:48, :], AF.Ln,
                                             bias=1.0)
                        nc.vector.tensor_scalar_mul(
                            dtld[64:80, :], dtld[0:16, :], NAE[:, 0:1])

                        # atot per chunk = exp(chunk-sums of ld)
                        red = psSm.tile([128, 32], f32, tag="sm", name="red")
                        nc.vector.tensor_reduce(
                            red[0:16, 0:16],
                            dtld[64:80, :].rearrange("p (c t) -> p c t", c=NCH),
                            op=OP.add, axis=mybir.AxisListType.X,
                        )
                        nc.scalar.activation(atot[:], red[0:16, 0:16], AF.Exp)
                        atT_ps = psSm.tile([128, 32], f32, tag="sm", name="atT_ps")
                        nc.tensor.transpose(
                            atT_ps[0:16, 0:16], atot[:], IDNF[0:16, 0:16])
                        nc.vector.tensor_copy(atotT[:], atT_ps[0:16, 0:16])
                        nc.sync.dma_start(
                            atotF[:].rearrange("p (c h) -> p c h", c=16), atotT[:])

                        # time-major dt/ld per chunk
                        for c in range(NCH):
                            trp = psT.tile([128, 96], f32, tag="trp", name="trp")
                            nc.tensor.transpose(
                                trp[:], dtld[:, c * T:(c + 1) * T], IDNF[0:96, 0:96])
                            nc.vector.tensor_copy(dtldT[:, c, 0:16], trp[:, 0:16])
                            nc.vector.tensor_copy(dtldT[:, c, 16:32], trp[:, 64:80])

                        # decay prep A: wdin/dtw/atb/S for all chunks
                        for c in range(NCH):
                            ld_bf = pW.tile([128, 16], bf16, tag="ld_bf")
                            nc.vector.tensor_copy(ld_bf[:], dtldT[:, c, 16:32])
                            wd_ps = psSm.tile([128, 48], f32, tag="sm", name="wd_ps")
                            nc.tensor.matmul(wd_ps[:, 0:16], ALOW[:], ld_bf[:],
                                             start=True, stop=True)
                            nc.tensor.matmul(wd_ps[:, 16:32], UINC[:], ld_bf[:],
                                             start=True, stop=True)
                            nc.scalar.activation(wdin_all[:, c, :], wd_ps[:, 0:32],
                                                 AF.Exp)
                            nc.vector.tensor_copy(TRall[:, c, 0:16], wd_ps[:, 16:32])
                            nc.vector.tensor_tensor(
                                dtw_all[:, c, :], dtldT[:, c, 0:16],
                                wdin_all[:, c, 0:16], op=OP.mult)
                            if c > 0:
                                nc.tensor.matmul(
                                    wd_ps[:, 32:48], ONESRF[:],
                                    atotF[0:1, c * 16:(c + 1) * 16],
                                    start=True, stop=True)
                                nc.vector.tensor_copy(atb_all[:, c, :],
                                                      wd_ps[:, 32:48])

                    if 8 <= m < 18:
                        conv_tile(m - 8)

            if dbg:
                nc.sync.dma_start(xbcc_o.ap(), xbc_c[:])
                nc.sync.dma_start(sz_o.ap(), sz[:])

            # ============ PHASE 2: chunked scan + gating + out_proj ============
            with (
                tc.tile_pool(name="pS", bufs=2) as pS,
                tc.tile_pool(name="pXbt", bufs=3) as pXbt,
                tc.tile_pool(name="pPrep", bufs=3) as pPrep,
                tc.tile_pool(name="pGn", bufs=2) as pGn,
                tc.tile_pool(name="pYT", bufs=2) as pYT,
                tc.tile_pool(name="psE", bufs=1, space="PSUM") as psE,
                tc.tile_pool(name="psY", bufs=1, space="PSUM") as psY,
                tc.tile_pool(name="psYI", bufs=1, space="PSUM") as psYI,
                tc.tile_pool(name="psS", bufs=1, space="PSUM") as psS,
            ):
                PS0 = {}
                PB4 = {}
                xbt = {}
                gn_tiles = []
                yTr = yT_d.ap().rearrange("(mo p) t -> p mo t", p=128)

                def out_group(tb, dm):
                    gtile = gn_tiles[tb]
                    po = psSm.tile([128, 512], f32, tag="sm",
                                   name=f"po{tb}_{dm}")
                    for k in range(8):
                        nc.tensor.matmul(
                            po[:], wo[:, k, dm * 128:(dm + 1) * 128],
                            gtile[:, k, :], start=(k == 0), stop=(k == 7))
                    yv = pYT.tile([128, 512], f32, tag="yT_sb")
                    nc.scalar.copy(yv[:], po[:])
                    nc.sync.dma_start(
                        yTr[:, dm, tb * 512:(tb + 1) * 512], yv[:])

                def prep(c):
                    # build P-matmul operands: S row-flat + B rows
                    lnt = pPrep.tile([128, 16], f32, tag="lnt")
                    nc.scalar.activation(lnt[:], dtldT[:, c, 0:16], AF.Ln,
                                         bias=0.0)
                    nc.vector.tensor_tensor(
                        TRall[:, c, 16:32], lnt[:], TRall[:, c, 0:16],
                        op=OP.subtract)
                    trt_ps = psSm.tile([32, 128], f32, tag="sm", name="trt_ps")
                    nc.tensor.transpose(trt_ps[:], TRall[:, c, :], IDNF[:, :])
                    TRT = pPrep.tile([32, 128], f32, tag="TRT")
                    nc.scalar.copy(TRT[:], trt_ps[:])
                    ps0 = pPrep.tile([1, 2048], f32, tag="PS0")
                    nc.sync.dma_start(
                        ps0[:].rearrange("p (b i) -> p b i", b=16), TRT[0:16, :])
                    pb4 = pPrep.tile([4, 4, 128], f32, tag="PB4")
                    for blk in range(4):
                        nc.sync.dma_start(pb4[0:4, blk, :],
                                          TRT[16 + 4 * blk: 20 + 4 * blk, :])
                    PS0[c] = ps0
                    PB4[c] = pb4

                def xbt_load(c):
                    csl = slice(c * T, (c + 1) * T)
                    xb = pXbt.tile([128, 1152], bf16, tag="xbt")
                    nc.sync.dma_start_transpose(xb[:, 0:1024], rt_dram[0:1024, csl])
                    nc.sync.dma_start_transpose(xb[:, 1024:1152],
                                                rt_dram[1024:1152, csl])
                    xbt[c] = xb

                prep(0)
                prep(1)
                xbt_load(0)
                xbt_load(1)

                for c in range(NCH):
                    if c + 2 < NCH:
                        prep(c + 2)
                        xbt_load(c + 2)
                    csl = slice(c * T, (c + 1) * T)
                    xb = xbt.pop(c)
                    ps0 = PS0.pop(c)
                    pb4 = PB4.pop(c)

                    # gt = B^T C (channel-major operands)
                    gt_ps = psSm.tile([128, 128], f32, tag="sm", name="gt_ps")
                    nc.tensor.matmul(gt_ps[:], xbc_c[:, 8, csl], xbc_c[:, 9, csl],
                                     start=True, stop=True)
                    gt = pS.tile([128, 128], bf16, tag="gt")
                    nc.scalar.copy(gt[:], gt_ps[:])

                    y_ps = psY.tile([128, 8, T], f32, tag="y_ps")

                    # inter-Y first (time-major, scaled), so transposes can
                    # interleave with intra starts per region below
                    yw_half = []
                    if c > 0:
                        for hf in range(2):
                            yi_ps = psYI.tile([128, 8, HD], f32, tag="yi_ps")
                            nc.tensor.matmul(
                                yi_ps[:], xbc_c[:, 9, csl],
                                s_sb[(c + 1) % 2][:, hf * 8:(hf + 1) * 8, :],
                                start=True, stop=True)
                            yw = pS.tile([128, 512], bf16, tag="yw")
                            nc.vector.tensor_tensor(
                                yw[:].rearrange("p (h d) -> p h d", h=8), yi_ps[:],
                                wdin_all[:, c, 16 + hf * 8: 16 + (hf + 1) * 8, None]
                                .to_broadcast([128, 8, HD]),
                                op=OP.mult)
                            yw_half.append(yw)

                    for hb in range(4):
                        e_ps = psE.tile([128, 4, 128], f32, tag="e_ps")
                        nc.tensor.matmul(
                            e_ps[:], ONESRF[:], ps0[0:1, hb * 512:(hb + 1) * 512],
                            start=True, stop=False)
                        nc.tensor.matmul(
                            e_ps[:], pb4[0:4, hb, :], IND4[:],
                            start=False, stop=False, skip_group_check=True)
                        nc.tensor.matmul(
                            e_ps[:], IDNB[:], MINF4[:],
                            start=False, stop=True, skip_group_check=True)
                        e_sb = pS.tile([128, 4, 128], bf16, tag="e_sb")
                        nc.scalar.activation(e_sb[:], e_ps[:], AF.Exp)
                        m_sb = pS.tile([128, 4, 128], bf16, tag="m_sb")
                        nc.vector.tensor_tensor(
                            m_sb[:], gt[:, None, :].to_broadcast([128, 4, 128]),
                            e_sb[:], op=OP.mult)
                        for hq in range(4):
                            h = hb * 4 + hq
                            ph, fh = (h % 2) * 64, h // 2
                            nc.tensor.matmul(
                                y_ps[ph:ph + 64, fh, :],
                                xb[:, h * HD:(h + 1) * HD], m_sb[:, hq, :],
                                start=True, stop=(c == 0),
                                skip_group_check=True)
                            if c > 0:
                                # accumulate this head's inter contribution
                                # before the next start=True clears the bank's
                                # has_written bits
                                nc.tensor.matmul(
                                    y_ps[ph:ph + 64, fh, :],
                                    yw_half[h // 8][:, (h % 8) * HD:
                                                    (h % 8 + 1) * HD],
                                    IDNB[:],
                                    start=False, stop=True,
                                    skip_group_check=True)

                    # state for this chunk
                    xdtw = pS.tile([128, NH, HD], bf16, tag="xdtw")
                    nc.gpsimd.tensor_tensor(
                        xdtw[:], xb[:, 0:1024].rearrange("p (h d) -> p h d", h=NH),
                        dtw_all[:, c, :, None].to_broadcast([128, NH, HD]),
                        op=OP.mult)
                    s_ps = psS.tile([128, NH, HD], f32, tag="s_ps")
                    nc.tensor.matmul(s_ps[:, 0:8, :], xb[:, 1024:1152],
                                     xdtw[:, 0:8, :], start=True, stop=True)
                    nc.tensor.matmul(s_ps[:, 8:16, :], xb[:, 1024:1152],
                                     xdtw[:, 8:16, :], start=True, stop=True)
                    if c == 0:
                        nc.vector.tensor_copy(s_sb[0][:], s_ps[:])
                    else:
                        s_scaled = pS.tile([128, NH, HD], bf16, tag="s_scaled")
                        nc.gpsimd.tensor_tensor(
                            s_scaled[:], s_sb[(c + 1) % 2][:],
                            atb_all[:, c, :, None].to_broadcast([128, NH, HD]),
                            op=OP.mult)
                        nc.vector.tensor_tensor(
                            s_sb[c % 2][:], s_scaled[:], s_ps[:], op=OP.add)

                    # y evac + gating + rmsnorm for this chunk
                    y_ch = pS.tile([128, 8, T], bf16, tag="y_ch")
                    for t in range(8):
                        nc.vector.scalar_tensor_tensor(
                            y_ch[:, t, :], xbc_c[:, t, csl], DCOL[:, t:t + 1],
                            y_ps[:, t, :], op0=OP.mult, op1=OP.add)
                    g_ch = pS.tile([128, 8, T], bf16, tag="g_ch")
                    nc.vector.tensor_tensor(g_ch[:], y_ch[:], sz[:, :, csl],
                                            op=OP.mult)
                    g2 = pS.tile([128, 8, T], bf16, tag="g2")
                    nc.scalar.square(g2[:], g_ch[:])
                    ss_ps = psSm.tile([128, 32], f32, tag="sm", name="ss_ps")
                    for t in range(8):
                        nc.tensor.matmul(ss_ps[:, 0:1], g2[:, t, :], ONEC[:],
                                         start=(t == 0), stop=(t == 7))
                    lnv = pPrep.tile([128, 1], f32, tag="lnv")
                    nc.scalar.activation(lnv[:], ss_ps[:, 0:1], AF.Ln,
                                         bias=EPSC[:, 0:1], scale=1.0 / D_INNER)
                    rstd = pPrep.tile([128, 1], f32, tag="rstd")
                    nc.scalar.activation(rstd[:], lnv[:], AF.Exp, scale=-0.5)
                    rsT_ps = psSm.tile([1, 128], f32, tag="sm", name="rsT_ps")
                    nc.tensor.transpose(rsT_ps[:], rstd[:], IDNF[:, :])
                    rsT = pPrep.tile([1, 128], bf16, tag="rsT")
                    nc.scalar.copy(rsT[:], rsT_ps[:])
                    rb_ps = psSm.tile([128, 128], f32, tag="sm", name="rb_ps")
                    nc.tensor.matmul(rb_ps[:], ONESRB[:], rsT[:],
                                     start=True, stop=True)
                    rb_sb = pS.tile([128, 128], bf16, tag="rb_sb")
                    nc.scalar.copy(rb_sb[:], rb_ps[:])

                    if c % 4 == 0:
                        gn_tb = pGn.tile([128, 8, 512], bf16, tag="gn_tb",
                                         name=f"gn_tb{c // 4}")
                        gn_tiles.append(gn_tb)
                    nc.vector.tensor_tensor(
                        gn_tb[:, :, (c % 4) * T:(c % 4 + 1) * T], g_ch[:],
                        rb_sb[:, None, :].to_broadcast([128, 8, 128]),
                        op=OP.mult)

                    if dbg:
                        nc.sync.dma_start(y_o.ap()[:, :, csl], y_ch[:])
                        nc.sync.dma_start(
                            gn_o.ap()[:, :, csl],
                            gn_tb[:, :, (c % 4) * T:(c % 4 + 1) * T])
                        nc.sync.dma_start(
                            s_o.ap()[:, c, :],
                            s_sb[c % 2][:].rearrange("p a b -> p (a b)"))

                    # out_proj: one dm-group per chunk, round-robin over
                    # finished 512-column blocks
                    if c >= 3:
                        out_group((c - 3) // 4, (c - 3) % 4)

                for i in range(13, 16):
                    out_group(3, i - 12)

    _fix_waits(nc, mybir)

    return nc


def _fix_waits(nc, mybir):
    """This walrus build supports one sem-wait slot per instruction; hoist
    excess waits onto preceding NoOps on the same engine."""
    nwn = [0]
    for bb in nc.main_func.blocks:
        newl = []
        changed = False
        for inst in bb.instructions:
            si = inst.sync_info
            waits = list(si.on_wait) if (si and si.on_wait) else []
            if len(waits) > 1:
                imm = [w for w in waits if w.wait_reg is None]
                reg = [w for w in waits if w.wait_reg is not None]
                keep = (reg + imm)[:1]
                spill = [w for w in waits if w not in keep]
                assert not any(w.wait_reg is not None for w in spill), inst.name
                for w in spill:
                    nwn[0] += 1
                    nop = mybir.InstNoOp(name=f"I-wsplit-{nwn[0]}", ins=[], outs=[])
                    nop.engine = inst.engine
                    nop.sync_info = mybir.SyncInfo(on_wait=[w], on_update=[])
                    nc.register_instruction(nop)
                    newl.append(nop)
                si.on_wait = keep
                changed = True
            newl.append(inst)
        if changed:
            bb.instructions = newl
    return nc


def _get_program(dbg=False):
    key = "nc_dbg" if dbg else "nc"
    if key not in _CACHE:
        _CACHE[key] = _build_program(dbg=dbg)
    return _CACHE[key]


def _host_consts():
    if "consts" in _CACHE:
        return _CACHE["consts"]
    import ml_dtypes
    k = np.arange(128)
    alow = (k[:, None] > k[None, :]).astype(np.float32)      # [k > j]
    uinc = (k[:, None] <= k[None, :]).astype(np.float32)     # [k <= i]
    idn = np.eye(128, dtype=np.float32)
    ind4 = np.repeat((np.arange(4)[:, None] == np.arange(4)[None, :])
                     .astype(np.float32)[:, :, None], 128, axis=2).reshape(4, 512)
    consts = dict(
        alow=alow.astype(ml_dtypes.bfloat16),
        uinc=uinc.astype(ml_dtypes.bfloat16),
        idnb=idn.astype(ml_dtypes.bfloat16),
        idnf=idn,
        ones=np.ones((128, 1), ml_dtypes.bfloat16),
        onesrf=np.ones((1, 128), np.float32),
        onesrb=np.ones((1, 128), ml_dtypes.bfloat16),
        minf4=np.tile((k[:, None] > k[None, :]).astype(np.float32) * NEG_INF,
                      (1, 4)).astype(ml_dtypes.bfloat16),
        ind4=np.ascontiguousarray(ind4),
    )
    _CACHE["consts"] = consts
    return consts


def _core_inputs(x_seq, p):
    """x_seq: (L, D_MODEL) f32 (already flipped for bw); p: dict of params."""
    import ml_dtypes
    consts = _host_consts()
    dcol = p["D"].astype(np.float32).repeat(HD).reshape(8, 128).T.copy()
    convw = np.ascontiguousarray(
        p["conv_w"].astype(np.float32).reshape(4, 10, 128).transpose(2, 1, 0)
    )
    convb = np.ascontiguousarray(p["conv_b"].astype(np.float32).reshape(10, 128).T)
    w_out = (p["norm_w"].astype(np.float32)[:, None]
             * p["out_proj"].astype(np.float32))
    return dict(
        xT=np.ascontiguousarray(x_seq.T).astype(ml_dtypes.bfloat16),
        w_in=np.ascontiguousarray(p["in_proj"]).astype(ml_dtypes.bfloat16),
        w_out=np.ascontiguousarray(w_out).astype(ml_dtypes.bfloat16),
        convw=convw,
        convb=convb,
        dtb=p["dt_bias"].astype(np.float32).reshape(16, 1),
        nae=(-np.exp(p["A_log"].astype(np.float32))).reshape(16, 1),
        dcol=dcol,
        **consts,
    )


def kernel(**inputs):
    from concourse.bass_utils import run_bass_kernel_spmd

    nc = _get_program()
    x = np.asarray(inputs["x"], np.float32)
    mask = np.asarray(inputs["padding_mask"])

    def params(pre):
        names = ["in_proj", "conv_w", "conv_b", "dt_bias", "A_log", "D", "norm_w", "out_proj"]
        return {n: np.asarray(inputs[pre + n]) for n in names}

    pf, pb = params("fw_"), params("bw_")
    in_maps = []
    for b in range(B_SZ):
        in_maps.append(_core_inputs(x[b], pf))
    for b in range(B_SZ):
        in_maps.append(_core_inputs(x[b][::-1], pb))

    res = run_bass_kernel_spmd(nc, in_maps, core_ids=list(range(8)))
    out = np.empty((B_SZ, L, D_MODEL), np.float32)
    for b in range(B_SZ):
        yf = res.results[b]["yT"].T
        yb = res.results[B_SZ + b]["yT"].T[::-1]
        out[b] = yf + yb
    out[mask] = 0.0
    return out


# revision 28
# speedup vs baseline: 1.5794x; 1.5794x over previous
"""Bidirectional Mamba2 layer on 8 NeuronCores (v2).

Sharding: 8 cores = 4 batch elements x 2 directions (fw/bw). Each core runs
one full Mamba2 layer on one sequence; host flips bw sequences, adds fw+bw,
applies the padding mask.

Per-core kernel (L=2048, chunked SSD scan, T=128), redesigned from the
baseline for engine balance:
  1. in_proj channel-major matmuls; depthwise conv interleaved per channel
     tile so DVE conv overlaps PE in_proj.
  2. decay matrices via cumsum S = UINC@ld; P[j,(h,i)] = S_h[i] + (ln dt -
     S)_h[j] + mask built with 3 matmuls per 4-head block (rank-1 bcast,
     K=4 indicator, K=128 mask); one exp per block; m = gt_bcast * e.
  3. state: one broadcast-scaled xdtw; 2 N=512 matmuls per chunk.
  4. inter-Y computed time-major (C_cm^T @ S_prev), scaled by w16 broadcast,
     transpose-accumulated into channel-major y PSUM via identity matmuls.
  5. gating + RMSNorm per chunk during the scan (norm_w folded into w_out
     host-side); out_proj per 512-column block.
"""

import numpy as np

D_MODEL = 512
D_STATE = 128
NH = 16
HD = 64
D_INNER = 1024
D_XBC = 1280
D_IN = 2320
L = 2048
T = 128
NCH = L // T
B_SZ = 4
EPS = 1e-5
NEG_INF = -1e30

_CACHE = {}


def _patch_drain(tile, mybir, ScopedClock):
    # workaround: this walrus build rejects >2 sem waits per instruction;
    # spread the TileContext exit-drain waits across nop instructions.
    def _drain_and_barrier(self, tick_clock, wait_clock):
        nc_ = self.nc
        probe = nc_.sync.nop()
        wait_clock.add_sem_waits(
            probe.ins, ScopedClock({None: tick_clock.global_clock})
        )
        waits = list(probe.ins.sync_info.on_wait or [])
        if probe.ins.sync_info is not None:
            probe.ins.sync_info.on_wait = waits[:1]
            rest = waits[1:]
        else:
            rest = []
        for w in rest:
            n = nc_.sync.nop()
            if n.ins.sync_info is None:
                n.ins.sync_info = mybir.SyncInfo(on_wait=[w], on_update=[])
            else:
                n.ins.sync_info.on_wait = [w]
        nc_.sync.drain()
        nc_.all_engine_barrier()
        assert self.sems is not None
        popped = nc_._tile_sem_poison_stack.pop()
        assert popped is self._sem_poison
        nc_.clear_and_free_semaphores(list(self.sems.allocated().values()))
        nc_.all_engine_barrier()

    tile.TileContext._drain_and_barrier = _drain_and_barrier


def _build_program(dbg=False):
    import concourse.bass as bass
    import concourse.mybir as mybir
    import concourse.tile as tile
    from concourse.vector_clock import ScopedClock

    _patch_drain(tile, mybir, ScopedClock)

    f32 = mybir.dt.float32
    bf16 = mybir.dt.bfloat16
    AF = mybir.ActivationFunctionType
    OP = mybir.AluOpType

    nc = bass.Bass("TRN2", target_bir_lowering=False, debug=False)

    # ---------------- DRAM I/O ----------------
    xT_d = nc.dram_tensor("xT", [D_MODEL, L], bf16, kind="ExternalInput")
    w_in_d = nc.dram_tensor("w_in", [D_MODEL, D_IN], bf16, kind="ExternalInput")
    w_out_d = nc.dram_tensor("w_out", [D_INNER, D_MODEL], bf16, kind="ExternalInput")
    convw_d = nc.dram_tensor("convw", [128, 10, 4], f32, kind="ExternalInput")
    convb_d = nc.dram_tensor("convb", [128, 10], f32, kind="ExternalInput")
    dtb_d = nc.dram_tensor("dtb", [16, 1], f32, kind="ExternalInput")
    nae_d = nc.dram_tensor("nae", [16, 1], f32, kind="ExternalInput")  # -exp(A_log)
    dcol_d = nc.dram_tensor("dcol", [128, 8], f32, kind="ExternalInput")
    alow_d = nc.dram_tensor("alow", [128, 128], bf16, kind="ExternalInput")
    uinc_d = nc.dram_tensor("uinc", [128, 128], bf16, kind="ExternalInput")
    idnb_d = nc.dram_tensor("idnb", [128, 128], bf16, kind="ExternalInput")
    idnf_d = nc.dram_tensor("idnf", [128, 128], f32, kind="ExternalInput")
    ones_d = nc.dram_tensor("ones", [128, 1], bf16, kind="ExternalInput")
    onesrf_d = nc.dram_tensor("onesrf", [1, 128], f32, kind="ExternalInput")
    onesrb_d = nc.dram_tensor("onesrb", [1, 128], bf16, kind="ExternalInput")
    minf4_d = nc.dram_tensor("minf4", [128, 512], bf16, kind="ExternalInput")
    ind8_d = nc.dram_tensor("ind8", [8, 512], bf16, kind="ExternalInput")
    ones2_d = nc.dram_tensor("ones2", [2, 128], bf16, kind="ExternalInput")
    yT_d = nc.dram_tensor("yT", [D_MODEL, L], f32, kind="ExternalOutput")
    if dbg:
        xbcc_o = nc.dram_tensor("xbcc_o", [128, 10, L], bf16, kind="ExternalOutput")
        sz_o = nc.dram_tensor("sz_o", [128, 8, L], bf16, kind="ExternalOutput")
        y_o = nc.dram_tensor("y_o", [128, 8, L], bf16, kind="ExternalOutput")
        gn_o = nc.dram_tensor("gn_o", [128, 8, L], bf16, kind="ExternalOutput")
        s_o = nc.dram_tensor("s_o", [128, NCH, NH * HD], bf16, kind="ExternalOutput")

    with tile.TileContext(nc) as tc:
        with (
            tc.tile_pool(name="const", bufs=1) as cpool,
            tc.tile_pool(name="dram", bufs=1, space="DRAM") as dpool,
            tc.tile_pool(name="mid", bufs=1) as mid,
            tc.tile_pool(name="psSm", bufs=2, space="PSUM") as psSm,
        ):
            # ---------------- constants ----------------
            ALOW = cpool.tile([128, 128], bf16, tag="alow")
            nc.sync.dma_start(ALOW[:], alow_d.ap())
            UINC = cpool.tile([128, 128], bf16, tag="uinc")
            nc.sync.dma_start(UINC[:], uinc_d.ap())
            IDNB = cpool.tile([128, 128], bf16, tag="idnb")
            nc.sync.dma_start(IDNB[:], idnb_d.ap())
            IDNF = cpool.tile([128, 128], f32, tag="idnf")
            nc.sync.dma_start(IDNF[:], idnf_d.ap())
            ONEC = cpool.tile([128, 1], bf16, tag="ones")
            nc.sync.dma_start(ONEC[:], ones_d.ap())
            ONESRF = cpool.tile([1, 128], f32, tag="onesrf")
            nc.sync.dma_start(ONESRF[:], onesrf_d.ap())
            ONESRB = cpool.tile([1, 128], bf16, tag="onesrb")
            nc.sync.dma_start(ONESRB[:], onesrb_d.ap())
            MINF4 = cpool.tile([128, 512], bf16, tag="minf4")
            nc.sync.dma_start(MINF4[:], minf4_d.ap())
            IND8 = cpool.tile([8, 512], bf16, tag="ind8")
            nc.sync.dma_start(IND8[:], ind8_d.ap())
            ONES2 = cpool.tile([2, 128], bf16, tag="ones2")
            nc.sync.dma_start(ONES2[:], ones2_d.ap())
            CONVW = cpool.tile([128, 10, 4], f32, tag="convw")
            nc.sync.dma_start(CONVW[:], convw_d.ap())
            CONVB = cpool.tile([128, 10], f32, tag="convb")
            nc.sync.dma_start(CONVB[:], convb_d.ap())
            DTB = cpool.tile([16, 1], f32, tag="dtb")
            nc.sync.dma_start(DTB[:], dtb_d.ap())
            NAE = cpool.tile([16, 1], f32, tag="nae")
            nc.sync.dma_start(NAE[:], nae_d.ap())
            DCOL = cpool.tile([128, 8], f32, tag="dcol")
            nc.sync.dma_start(DCOL[:], dcol_d.ap())
            EPSC = cpool.tile([128, 1], f32, tag="epsc")
            nc.vector.memset(EPSC[:], EPS)

            # ---------------- persistent tensors ----------------
            dtldT = mid.tile([128, NCH, 32], f32, tag="dtldT")  # 0:16 dt, 16:32 ld
            TRall = mid.tile([128, NCH, 32], f32, tag="TRall")  # 0:16 S, 16:32 lndt-S
            atot = mid.tile([16, 16], f32, tag="atot")          # [head, chunk]
            atotT = mid.tile([16, 16], f32, tag="atotT")
            atotF = mid.tile([1, 256], f32, tag="atotF")
            wdin_all = mid.tile([128, NCH, 32], f32, tag="wdin_all")
            dtw_all = mid.tile([128, NCH, 16], f32, tag="dtw_all")
            atb_all = mid.tile([128, NCH, 16], f32, tag="atb_all")
            s_sb = [mid.tile([128, NH, HD], bf16, tag=f"s_sb{i}", name=f"s_sb{i}")
                    for i in range(2)]
            xbc_c = mid.tile([128, 10, L], bf16, tag="xbc_c")
            sz = mid.tile([128, 8, L], bf16, tag="sz")
            wo = mid.tile([128, 8, D_MODEL], bf16, tag="wo")
            nc.sync.dma_start(
                wo[:], w_out_d.ap().rearrange("(ko p) m -> p ko m", p=128))

            rt_dram = dpool.tile([1152, L], bf16)

            # ============ PHASE 1: in_proj + conv + dt pipeline ============
            with (
                tc.tile_pool(name="pA", bufs=1) as pA,
                tc.tile_pool(name="pW", bufs=3) as pW,
                tc.tile_pool(name="pC", bufs=2) as pC,
                tc.tile_pool(name="ps1", bufs=4, space="PSUM") as ps1,
                tc.tile_pool(name="psT", bufs=2, space="PSUM") as psT,
            ):
                dtld = pA.tile([96, L], f32, tag="dtld")  # 0:16 dt, 32:48 scr, 64:80 ld
                xTs = pA.tile([128, 4, L], bf16, tag="xTs")
                xbc_pre = pA.tile([128, 10, L + 3], bf16, tag="xbc_pre")
                xTr = xT_d.ap().rearrange("(ko p) t -> p ko t", p=128)
                wir = w_in_d.ap().rearrange("(ko p) m -> p ko m", p=128)
                for k in range(4):
                    nc.sync.dma_start(xTs[:, k, :], xTr[:, k, :])
                nc.vector.memset(xbc_pre[:, :, 0:3], 0.0)

                def conv_tile(t):
                    acc = pC.tile([128, L], bf16, tag="conv_acc")
                    nc.vector.tensor_scalar_mul(
                        acc[:], xbc_pre[:, t, 0:L], CONVW[:, t, 0:1])
                    for k in range(1, 4):
                        nc.vector.scalar_tensor_tensor(
                            acc[:], xbc_pre[:, t, k:k + L],
                            CONVW[:, t, k:k + 1], acc[:],
                            op0=OP.mult, op1=OP.add,
                        )
                    nc.scalar.activation(
                        xbc_c[:, t, :], acc[:], AF.Silu, bias=CONVB[:, t:t + 1])
                    if t < 9:
                        nc.sync.dma_start(
                            rt_dram[t * 128:(t + 1) * 128, :], xbc_c[:, t, :])

                for m in [18] + list(range(8, 18)) + list(range(0, 8)):
                    mp = 128 if m < 18 else 16
                    wis = pW.tile([128, 4, 128], bf16, tag="wis")
                    for k in range(4):
                        nc.sync.dma_start(wis[:, k, 0:mp], wir[:, k, m * 128:m * 128 + mp])
                    for tb in range(4):
                        tsl = slice(tb * 512, (tb + 1) * 512)
                        ps = ps1.tile([128, 512], f32, tag="ps_inproj")
                        for k in range(4):
                            nc.tensor.matmul(
                                ps[:mp, :], wis[:, k, 0:mp], xTs[:, k, tsl],
                                start=(k == 0), stop=(k == 3),
                            )
                        if m < 8:
                            nc.scalar.activation(sz[:, m, tsl], ps[:, :], AF.Silu)
                        elif m < 18:
                            nc.scalar.copy(
                                xbc_pre[:, m - 8, 3 + tb * 512: 3 + (tb + 1) * 512],
                                ps[:, :])
                        else:
                            nc.scalar.copy(dtld[32:48, tsl], ps[:16, :])

                    if m == 18:
                        # dt = softplus(pre + dtb); ld = -exp(A_log) * dt
                        nc.scalar.activation(dtld[32:48, :], dtld[32:48, :], AF.Exp,
                                             bias=DTB[:, 0:1])
                        nc.scalar.activation(dtld[0:16, :], dtld[32:48, :], AF.Ln,
                                             bias=1.0)
                        nc.vector.tensor_scalar_mul(
                            dtld[64:80, :], dtld[0:16, :], NAE[:, 0:1])

                        # atot per chunk = exp(chunk-sums of ld)
                        red = psSm.tile([128, 32], f32, tag="sm", name="red")
                        nc.vector.tensor_reduce(
                            red[0:16, 0:16],
                            dtld[64:80, :].rearrange("p (c t) -> p c t", c=NCH),
                            op=OP.add, axis=mybir.AxisListType.X,
                        )
                        nc.scalar.activation(atot[:], red[0:16, 0:16], AF.Exp)
                        atT_ps = psSm.tile([128, 32], f32, tag="sm", name="atT_ps")
                        nc.tensor.transpose(
                            atT_ps[0:16, 0:16], atot[:], IDNF[0:16, 0:16])
                        nc.vector.tensor_copy(atotT[:], atT_ps[0:16, 0:16])
                        nc.sync.dma_start(
                            atotF[:].rearrange("p (c h) -> p c h", c=16), atotT[:])

                        # time-major dt/ld per chunk
                        for c in range(NCH):
                            trp = psT.tile([128, 96], f32, tag="trp", name="trp")
                            nc.tensor.transpose(
                                trp[:], dtld[:, c * T:(c + 1) * T], IDNF[0:96, 0:96])
                            nc.vector.tensor_copy(dtldT[:, c, 0:16], trp[:, 0:16])
                            nc.vector.tensor_copy(dtldT[:, c, 16:32], trp[:, 64:80])

                        # decay prep A: wdin/dtw/atb/S for all chunks
                        for c in range(NCH):
                            ld_bf = pW.tile([128, 16], bf16, tag="ld_bf")
                            nc.vector.tensor_copy(ld_bf[:], dtldT[:, c, 16:32])
                            wd_ps = psSm.tile([128, 48], f32, tag="sm", name="wd_ps")
                            nc.tensor.matmul(wd_ps[:, 0:16], ALOW[:], ld_bf[:],
                                             start=True, stop=True)
                            nc.tensor.matmul(wd_ps[:, 16:32], UINC[:], ld_bf[:],
                                             start=True, stop=True)
                            nc.scalar.activation(wdin_all[:, c, :], wd_ps[:, 0:32],
                                                 AF.Exp)
                            nc.vector.tensor_copy(TRall[:, c, 0:16], wd_ps[:, 16:32])
                            nc.vector.tensor_tensor(
                                dtw_all[:, c, :], dtldT[:, c, 0:16],
                                wdin_all[:, c, 0:16], op=OP.mult)
                            if c > 0:
                                nc.tensor.matmul(
                                    wd_ps[:, 32:48], ONESRF[:],
                                    atotF[0:1, c * 16:(c + 1) * 16],
                                    start=True, stop=True)
                                nc.vector.tensor_copy(atb_all[:, c, :],
                                                      wd_ps[:, 32:48])

                    if 8 <= m < 18:
                        conv_tile(m - 8)

            if dbg:
                nc.sync.dma_start(xbcc_o.ap(), xbc_c[:])
                nc.sync.dma_start(sz_o.ap(), sz[:])

            # ============ PHASE 2: chunked scan + gating + out_proj ============
            with (
                tc.tile_pool(name="pS", bufs=2) as pS,
                tc.tile_pool(name="pXbt", bufs=3) as pXbt,
                tc.tile_pool(name="pPrep", bufs=3) as pPrep,
                tc.tile_pool(name="pGn", bufs=2) as pGn,
                tc.tile_pool(name="pYT", bufs=2) as pYT,
                tc.tile_pool(name="psE", bufs=1, space="PSUM") as psE,
                tc.tile_pool(name="psY", bufs=1, space="PSUM") as psY,
                tc.tile_pool(name="psYI", bufs=1, space="PSUM") as psYI,
                tc.tile_pool(name="psS", bufs=1, space="PSUM") as psS,
            ):
                PS0 = {}
                PB4 = {}
                xbt = {}
                gn_tiles = []
                yTr = yT_d.ap().rearrange("(mo p) t -> p mo t", p=128)

                def out_group(tb, dm):
                    gtile = gn_tiles[tb]
                    po = psSm.tile([128, 512], f32, tag="sm",
                                   name=f"po{tb}_{dm}")
                    for k in range(8):
                        nc.tensor.matmul(
                            po[:], wo[:, k, dm * 128:(dm + 1) * 128],
                            gtile[:, k, :], start=(k == 0), stop=(k == 7))
                    yv = pYT.tile([128, 512], f32, tag="yT_sb")
                    nc.scalar.copy(yv[:], po[:])
                    nc.sync.dma_start(
                        yTr[:, dm, tb * 512:(tb + 1) * 512], yv[:])

                def prep(c):
                    # build P-matmul operands: S row-flat + B rows
                    lnt = pPrep.tile([128, 16], f32, tag="lnt")
                    nc.scalar.activation(lnt[:], dtldT[:, c, 0:16], AF.Ln,
                                         bias=0.0)
                    nc.vector.tensor_tensor(
                        TRall[:, c, 16:32], lnt[:], TRall[:, c, 0:16],
                        op=OP.subtract)
                    trt_ps = psSm.tile([32, 128], f32, tag="sm", name="trt_ps")
                    nc.tensor.transpose(trt_ps[:], TRall[:, c, :], IDNF[:, :])
                    TRT = pPrep.tile([32, 128], f32, tag="TRT")
                    nc.scalar.copy(TRT[:], trt_ps[:])
                    # split into bf16 hi+lo so the P matmuls run in bf16 mode
                    TRTh = pPrep.tile([32, 128], bf16, tag="TRTh")
                    nc.vector.tensor_copy(TRTh[:], TRT[:])
                    TRTl = pPrep.tile([32, 128], bf16, tag="TRTl")
                    nc.vector.tensor_tensor(TRTl[:], TRT[:], TRTh[:],
                                            op=OP.subtract)
                    ps0 = pPrep.tile([2, 2048], bf16, tag="PS0")
                    nc.sync.dma_start(
                        ps0[0:1, :].rearrange("p (b i) -> p b i", b=16),
                        TRTh[0:16, :])
                    nc.sync.dma_start(
                        ps0[1:2, :].rearrange("p (b i) -> p b i", b=16),
                        TRTl[0:16, :])
                    pb4 = pPrep.tile([8, 4, 128], bf16, tag="PB4")
                    for blk in range(4):
                        nc.sync.dma_start(pb4[0:4, blk, :],
                                          TRTh[16 + 4 * blk: 20 + 4 * blk, :])
                        nc.sync.dma_start(pb4[4:8, blk, :],
                                          TRTl[16 + 4 * blk: 20 + 4 * blk, :])
                    PS0[c] = ps0
                    PB4[c] = pb4

                def xbt_load(c):
                    csl = slice(c * T, (c + 1) * T)
                    xb = pXbt.tile([128, 1152], bf16, tag="xbt")
                    nc.sync.dma_start_transpose(xb[:, 0:1024], rt_dram[0:1024, csl])
                    nc.sync.dma_start_transpose(xb[:, 1024:1152],
                                                rt_dram[1024:1152, csl])
                    xbt[c] = xb

                prep(0)
                prep(1)
                xbt_load(0)
                xbt_load(1)

                for c in range(NCH):
                    if c + 2 < NCH:
                        prep(c + 2)
                        xbt_load(c + 2)
                    csl = slice(c * T, (c + 1) * T)
                    xb = xbt.pop(c)
                    ps0 = PS0.pop(c)
                    pb4 = PB4.pop(c)

                    # gt = B^T C (channel-major operands)
                    gt_ps = psSm.tile([128, 128], f32, tag="sm", name="gt_ps")
                    nc.tensor.matmul(gt_ps[:], xbc_c[:, 8, csl], xbc_c[:, 9, csl],
                                     start=True, stop=True)
                    gt = pS.tile([128, 128], bf16, tag="gt")
                    nc.scalar.copy(gt[:], gt_ps[:])

                    y_ps = psY.tile([128, 8, T], f32, tag="y_ps")

                    # inter-Y first (time-major, scaled), so transposes can
                    # interleave with intra starts per region below
                    yw_half = []
                    if c > 0:
                        for hf in range(2):
                            yi_ps = psYI.tile([128, 8, HD], f32, tag="yi_ps")
                            nc.tensor.matmul(
                                yi_ps[:], xbc_c[:, 9, csl],
                                s_sb[(c + 1) % 2][:, hf * 8:(hf + 1) * 8, :],
                                start=True, stop=True)
                            yw = pS.tile([128, 512], bf16, tag="yw")
                            nc.vector.tensor_tensor(
                                yw[:].rearrange("p (h d) -> p h d", h=8), yi_ps[:],
                                wdin_all[:, c, 16 + hf * 8: 16 + (hf + 1) * 8, None]
                                .to_broadcast([128, 8, HD]),
                                op=OP.mult)
                            yw_half.append(yw)

                    for hb in range(4):
                        e_ps = psE.tile([128, 4, 128], f32, tag="e_ps")
                        nc.tensor.matmul(
                            e_ps[:], ONES2[:], ps0[0:2, hb * 512:(hb + 1) * 512],
                            start=True, stop=False)
                        nc.tensor.matmul(
                            e_ps[:], pb4[0:8, hb, :], IND8[:],
                            start=False, stop=False, skip_group_check=True)
                        nc.tensor.matmul(
                            e_ps[:], IDNB[:], MINF4[:],
                            start=False, stop=True, skip_group_check=True)
                        e_sb = pS.tile([128, 4, 128], bf16, tag="e_sb")
                        nc.scalar.activation(e_sb[:], e_ps[:], AF.Exp)
                        m_sb = pS.tile([128, 4, 128], bf16, tag="m_sb")
                        nc.vector.tensor_tensor(
                            m_sb[:], gt[:, None, :].to_broadcast([128, 4, 128]),
                            e_sb[:], op=OP.mult)
                        for hq in range(4):
                            h = hb * 4 + hq
                            ph, fh = (h % 2) * 64, h // 2
                            nc.tensor.matmul(
                                y_ps[ph:ph + 64, fh, :],
                                xb[:, h * HD:(h + 1) * HD], m_sb[:, hq, :],
                                start=True, stop=(c == 0),
                                skip_group_check=True)
                            if c > 0:
                                # accumulate this head's inter contribution
                                # before the next start=True clears the bank's
                                # has_written bits
                                nc.tensor.matmul(
                                    y_ps[ph:ph + 64, fh, :],
                                    yw_half[h // 8][:, (h % 8) * HD:
                                                    (h % 8 + 1) * HD],
                                    IDNB[:],
                                    start=False, stop=True,
                                    skip_group_check=True)

                    # state for this chunk
                    xdtw = pS.tile([128, NH, HD], bf16, tag="xdtw")
                    nc.gpsimd.tensor_tensor(
                        xdtw[:], xb[:, 0:1024].rearrange("p (h d) -> p h d", h=NH),
                        dtw_all[:, c, :, None].to_broadcast([128, NH, HD]),
                        op=OP.mult)
                    s_ps = psS.tile([128, NH, HD], f32, tag="s_ps")
                    nc.tensor.matmul(s_ps[:, 0:8, :], xb[:, 1024:1152],
                                     xdtw[:, 0:8, :], start=True, stop=True)
                    nc.tensor.matmul(s_ps[:, 8:16, :], xb[:, 1024:1152],
                                     xdtw[:, 8:16, :], start=True, stop=True)
                    if c == 0:
                        nc.vector.tensor_copy(s_sb[0][:], s_ps[:])
                    else:
                        s_scaled = pS.tile([128, NH, HD], bf16, tag="s_scaled")
                        nc.gpsimd.tensor_tensor(
                            s_scaled[:], s_sb[(c + 1) % 2][:],
                            atb_all[:, c, :, None].to_broadcast([128, NH, HD]),
                            op=OP.mult)
                        nc.vector.tensor_tensor(
                            s_sb[c % 2][:], s_scaled[:], s_ps[:], op=OP.add)

                    # y evac + gating + rmsnorm for this chunk
                    y_ch = pS.tile([128, 8, T], bf16, tag="y_ch")
                    for t in range(8):
                        nc.vector.scalar_tensor_tensor(
                            y_ch[:, t, :], xbc_c[:, t, csl], DCOL[:, t:t + 1],
                            y_ps[:, t, :], op0=OP.mult, op1=OP.add)
                    g_ch = pS.tile([128, 8, T], bf16, tag="g_ch")
                    nc.vector.tensor_tensor(g_ch[:], y_ch[:], sz[:, :, csl],
                                            op=OP.mult)
                    g2 = pS.tile([128, 8, T], bf16, tag="g2")
                    nc.scalar.square(g2[:], g_ch[:])
                    ss_ps = psSm.tile([128, 32], f32, tag="sm", name="ss_ps")
                    for t in range(8):
                        nc.tensor.matmul(ss_ps[:, 0:1], g2[:, t, :], ONEC[:],
                                         start=(t == 0), stop=(t == 7))
                    lnv = pPrep.tile([128, 1], f32, tag="lnv")
                    nc.scalar.activation(lnv[:], ss_ps[:, 0:1], AF.Ln,
                                         bias=EPSC[:, 0:1], scale=1.0 / D_INNER)
                    rstd = pPrep.tile([128, 1], f32, tag="rstd")
                    nc.scalar.activation(rstd[:], lnv[:], AF.Exp, scale=-0.5)
                    rsT_ps = psSm.tile([1, 128], f32, tag="sm", name="rsT_ps")
                    nc.tensor.transpose(rsT_ps[:], rstd[:], IDNF[:, :])
                    rsT = pPrep.tile([1, 128], bf16, tag="rsT")
                    nc.scalar.copy(rsT[:], rsT_ps[:])
                    rb_ps = psSm.tile([128, 128], f32, tag="sm", name="rb_ps")
                    nc.tensor.matmul(rb_ps[:], ONESRB[:], rsT[:],
                                     start=True, stop=True)
                    rb_sb = pS.tile([128, 128], bf16, tag="rb_sb")
                    nc.scalar.copy(rb_sb[:], rb_ps[:])

                    if c % 4 == 0:
                        gn_tb = pGn.tile([128, 8, 512], bf16, tag="gn_tb",
                                         name=f"gn_tb{c // 4}")
                        gn_tiles.append(gn_tb)
                    nc.vector.tensor_tensor(
                        gn_tb[:, :, (c % 4) * T:(c % 4 + 1) * T], g_ch[:],
                        rb_sb[:, None, :].to_broadcast([128, 8, 128]),
                        op=OP.mult)

                    if dbg:
                        nc.sync.dma_start(y_o.ap()[:, :, csl], y_ch[:])
                        nc.sync.dma_start(
                            gn_o.ap()[:, :, csl],
                            gn_tb[:, :, (c % 4) * T:(c % 4 + 1) * T])
                        nc.sync.dma_start(
                            s_o.ap()[:, c, :],
                            s_sb[c % 2][:].rearrange("p a b -> p (a b)"))

                    # out_proj: one dm-group per chunk, round-robin over
                    # finished 512-column blocks
                    if c >= 3:
                        out_group((c - 3) // 4, (c - 3) % 4)

                for i in range(13, 16):
                    out_group(3, i - 12)

    _fix_waits(nc, mybir)

    return nc


def _fix_waits(nc, mybir):
    """This walrus build supports one sem-wait slot per instruction; hoist
    excess waits onto preceding NoOps on the same engine."""
    nwn = [0]
    for bb in nc.main_func.blocks:
        newl = []
        changed = False
        for inst in bb.instructions:
            si = inst.sync_info
            waits = list(si.on_wait) if (si and si.on_wait) else []
            if len(waits) > 1:
                imm = [w for w in waits if w.wait_reg is None]
                reg = [w for w in waits if w.wait_reg is not None]
                keep = (reg + imm)[:1]
                spill = [w for w in waits if w not in keep]
                assert not any(w.wait_reg is not None for w in spill), inst.name
                for w in spill:
                    nwn[0] += 1
                    nop = mybir.InstNoOp(name=f"I-wsplit-{nwn[0]}", ins=[], outs=[])
                    nop.engine = inst.engine
                    nop.sync_info = mybir.SyncInfo(on_wait=[w], on_update=[])
                    nc.register_instruction(nop)
                    newl.append(nop)
                si.on_wait = keep
                changed = True
            newl.append(inst)
        if changed:
            bb.instructions = newl
    return nc


def _get_program(dbg=False):
    key = "nc_dbg" if dbg else "nc"
    if key not in _CACHE:
        _CACHE[key] = _build_program(dbg=dbg)
    return _CACHE[key]


def _host_consts():
    if "consts" in _CACHE:
        return _CACHE["consts"]
    import ml_dtypes
    k = np.arange(128)
    alow = (k[:, None] > k[None, :]).astype(np.float32)      # [k > j]
    uinc = (k[:, None] <= k[None, :]).astype(np.float32)     # [k <= i]
    idn = np.eye(128, dtype=np.float32)
    ind4 = np.repeat((np.arange(4)[:, None] == np.arange(4)[None, :])
                     .astype(np.float32)[:, :, None], 128, axis=2).reshape(4, 512)
    consts = dict(
        ind8=np.tile(ind4, (2, 1)).astype(ml_dtypes.bfloat16),
        ones2=np.ones((2, 128), ml_dtypes.bfloat16),
        alow=alow.astype(ml_dtypes.bfloat16),
        uinc=uinc.astype(ml_dtypes.bfloat16),
        idnb=idn.astype(ml_dtypes.bfloat16),
        idnf=idn,
        ones=np.ones((128, 1), ml_dtypes.bfloat16),
        onesrf=np.ones((1, 128), np.float32),
        onesrb=np.ones((1, 128), ml_dtypes.bfloat16),
        minf4=np.tile((k[:, None] > k[None, :]).astype(np.float32) * NEG_INF,
                      (1, 4)).astype(ml_dtypes.bfloat16),
    )
    _CACHE["consts"] = consts
    return consts


def _core_inputs(x_seq, p):
    """x_seq: (L, D_MODEL) f32 (already flipped for bw); p: dict of params."""
    import ml_dtypes
    consts = _host_consts()
    dcol = p["D"].astype(np.float32).repeat(HD).reshape(8, 128).T.copy()
    convw = np.ascontiguousarray(
        p["conv_w"].astype(np.float32).reshape(4, 10, 128).transpose(2, 1, 0)
    )
    convb = np.ascontiguousarray(p["conv_b"].astype(np.float32).reshape(10, 128).T)
    w_out = (p["norm_w"].astype(np.float32)[:, None]
             * p["out_proj"].astype(np.float32))
    return dict(
        xT=np.ascontiguousarray(x_seq.T).astype(ml_dtypes.bfloat16),
        w_in=np.ascontiguousarray(p["in_proj"]).astype(ml_dtypes.bfloat16),
        w_out=np.ascontiguousarray(w_out).astype(ml_dtypes.bfloat16),
        convw=convw,
        convb=convb,
        dtb=p["dt_bias"].astype(np.float32).reshape(16, 1),
        nae=(-np.exp(p["A_log"].astype(np.float32))).reshape(16, 1),
        dcol=dcol,
        **consts,
    )


def kernel(**inputs):
    from concourse.bass_utils import run_bass_kernel_spmd

    nc = _get_program()
    x = np.asarray(inputs["x"], np.float32)
    mask = np.asarray(inputs["padding_mask"])

    def params(pre):
        names = ["in_proj", "conv_w", "conv_b", "dt_bias", "A_log", "D", "norm_w", "out_proj"]
        return {n: np.asarray(inputs[pre + n]) for n in names}

    pf, pb = params("fw_"), params("bw_")
    in_maps = []
    for b in range(B_SZ):
        in_maps.append(_core_inputs(x[b], pf))
    for b in range(B_SZ):
        in_maps.append(_core_inputs(x[b][::-1], pb))

    res = run_bass_kernel_spmd(nc, in_maps, core_ids=list(range(8)))
    out = np.empty((B_SZ, L, D_MODEL), np.float32)
    for b in range(B_SZ):
        yf = res.results[b]["yT"].T
        yb = res.results[B_SZ + b]["yT"].T[::-1]
        out[b] = yf + yb
    out[mask] = 0.0
    return out


# revision 35
# speedup vs baseline: 1.5934x; 1.0089x over previous
"""Bidirectional Mamba2 layer on 8 NeuronCores (v2).

Sharding: 8 cores = 4 batch elements x 2 directions (fw/bw). Each core runs
one full Mamba2 layer on one sequence; host flips bw sequences, adds fw+bw,
applies the padding mask.

Per-core kernel (L=2048, chunked SSD scan, T=128), redesigned from the
baseline for engine balance:
  1. in_proj channel-major matmuls; depthwise conv interleaved per channel
     tile so DVE conv overlaps PE in_proj.
  2. decay matrices via cumsum S = UINC@ld; P[j,(h,i)] = S_h[i] + (ln dt -
     S)_h[j] + mask built with 3 matmuls per 4-head block (rank-1 bcast,
     K=4 indicator, K=128 mask); one exp per block; m = gt_bcast * e.
  3. state: one broadcast-scaled xdtw; 2 N=512 matmuls per chunk.
  4. inter-Y computed time-major (C_cm^T @ S_prev), scaled by w16 broadcast,
     transpose-accumulated into channel-major y PSUM via identity matmuls.
  5. gating + RMSNorm per chunk during the scan (norm_w folded into w_out
     host-side); out_proj per 512-column block.
"""

import numpy as np

D_MODEL = 512
D_STATE = 128
NH = 16
HD = 64
D_INNER = 1024
D_XBC = 1280
D_IN = 2320
L = 2048
T = 128
NCH = L // T
B_SZ = 4
EPS = 1e-5
NEG_INF = -1e30

_CACHE = {}


def _patch_drain(tile, mybir, ScopedClock):
    # workaround: this walrus build rejects >2 sem waits per instruction;
    # spread the TileContext exit-drain waits across nop instructions.
    def _drain_and_barrier(self, tick_clock, wait_clock):
        nc_ = self.nc
        probe = nc_.sync.nop()
        wait_clock.add_sem_waits(
            probe.ins, ScopedClock({None: tick_clock.global_clock})
        )
        waits = list(probe.ins.sync_info.on_wait or [])
        if probe.ins.sync_info is not None:
            probe.ins.sync_info.on_wait = waits[:1]
            rest = waits[1:]
        else:
            rest = []
        for w in rest:
            n = nc_.sync.nop()
            if n.ins.sync_info is None:
                n.ins.sync_info = mybir.SyncInfo(on_wait=[w], on_update=[])
            else:
                n.ins.sync_info.on_wait = [w]
        nc_.sync.drain()
        nc_.all_engine_barrier()
        assert self.sems is not None
        popped = nc_._tile_sem_poison_stack.pop()
        assert popped is self._sem_poison
        nc_.clear_and_free_semaphores(list(self.sems.allocated().values()))
        nc_.all_engine_barrier()

    tile.TileContext._drain_and_barrier = _drain_and_barrier


def _build_program(dbg=False):
    import concourse.bass as bass
    import concourse.mybir as mybir
    import concourse.tile as tile
    from concourse.vector_clock import ScopedClock

    _patch_drain(tile, mybir, ScopedClock)

    f32 = mybir.dt.float32
    bf16 = mybir.dt.bfloat16
    AF = mybir.ActivationFunctionType
    OP = mybir.AluOpType

    nc = bass.Bass("TRN2", target_bir_lowering=False, debug=False)

    # ---------------- DRAM I/O ----------------
    xT_d = nc.dram_tensor("xT", [D_MODEL, L], bf16, kind="ExternalInput")
    w_in_d = nc.dram_tensor("w_in", [D_MODEL, D_IN], bf16, kind="ExternalInput")
    w_out_d = nc.dram_tensor("w_out", [D_INNER, D_MODEL], bf16, kind="ExternalInput")
    convw_d = nc.dram_tensor("convw", [128, 10, 4], f32, kind="ExternalInput")
    convb_d = nc.dram_tensor("convb", [128, 10], f32, kind="ExternalInput")
    dtb_d = nc.dram_tensor("dtb", [16, 1], f32, kind="ExternalInput")
    nae_d = nc.dram_tensor("nae", [16, 1], f32, kind="ExternalInput")  # -exp(A_log)
    dcol_d = nc.dram_tensor("dcol", [128, 8], f32, kind="ExternalInput")
    alow_d = nc.dram_tensor("alow", [128, 128], bf16, kind="ExternalInput")
    uinc_d = nc.dram_tensor("uinc", [128, 128], bf16, kind="ExternalInput")
    idnb_d = nc.dram_tensor("idnb", [128, 128], bf16, kind="ExternalInput")
    idnf_d = nc.dram_tensor("idnf", [128, 128], f32, kind="ExternalInput")
    ones_d = nc.dram_tensor("ones", [128, 1], bf16, kind="ExternalInput")
    onesrf_d = nc.dram_tensor("onesrf", [1, 128], f32, kind="ExternalInput")
    onesrb_d = nc.dram_tensor("onesrb", [1, 128], bf16, kind="ExternalInput")
    minf4_d = nc.dram_tensor("minf4", [128, 512], bf16, kind="ExternalInput")
    ind8_d = nc.dram_tensor("ind8", [8, 512], bf16, kind="ExternalInput")
    ones2_d = nc.dram_tensor("ones2", [2, 128], bf16, kind="ExternalInput")
    yT_d = nc.dram_tensor("yT", [D_MODEL, L], f32, kind="ExternalOutput")
    if dbg:
        xbcc_o = nc.dram_tensor("xbcc_o", [128, 10, L], bf16, kind="ExternalOutput")
        sz_o = nc.dram_tensor("sz_o", [128, 8, L], bf16, kind="ExternalOutput")
        y_o = nc.dram_tensor("y_o", [128, 8, L], bf16, kind="ExternalOutput")
        gn_o = nc.dram_tensor("gn_o", [128, 8, L], bf16, kind="ExternalOutput")
        s_o = nc.dram_tensor("s_o", [128, NCH, NH * HD], bf16, kind="ExternalOutput")

    with tile.TileContext(nc) as tc:
        with (
            tc.tile_pool(name="const", bufs=1) as cpool,
            tc.tile_pool(name="dram", bufs=1, space="DRAM") as dpool,
            tc.tile_pool(name="mid", bufs=1) as mid,
        ):
            # ---------------- constants ----------------
            ALOW = cpool.tile([128, 128], bf16, tag="alow")
            nc.sync.dma_start(ALOW[:], alow_d.ap())
            UINC = cpool.tile([128, 128], bf16, tag="uinc")
            nc.sync.dma_start(UINC[:], uinc_d.ap())
            IDNB = cpool.tile([128, 128], bf16, tag="idnb")
            nc.sync.dma_start(IDNB[:], idnb_d.ap())
            IDNF = cpool.tile([128, 128], f32, tag="idnf")
            nc.sync.dma_start(IDNF[:], idnf_d.ap())
            ONEC = cpool.tile([128, 1], bf16, tag="ones")
            nc.sync.dma_start(ONEC[:], ones_d.ap())
            ONESRF = cpool.tile([1, 128], f32, tag="onesrf")
            nc.sync.dma_start(ONESRF[:], onesrf_d.ap())
            ONESRB = cpool.tile([1, 128], bf16, tag="onesrb")
            nc.sync.dma_start(ONESRB[:], onesrb_d.ap())
            MINF4 = cpool.tile([128, 512], bf16, tag="minf4")
            nc.sync.dma_start(MINF4[:], minf4_d.ap())
            IND8 = cpool.tile([8, 512], bf16, tag="ind8")
            nc.sync.dma_start(IND8[:], ind8_d.ap())
            ONES2 = cpool.tile([2, 128], bf16, tag="ones2")
            nc.sync.dma_start(ONES2[:], ones2_d.ap())
            CONVW = cpool.tile([128, 10, 4], f32, tag="convw")
            nc.sync.dma_start(CONVW[:], convw_d.ap())
            CONVB = cpool.tile([128, 10], f32, tag="convb")
            nc.sync.dma_start(CONVB[:], convb_d.ap())
            DTB = cpool.tile([16, 1], f32, tag="dtb")
            nc.sync.dma_start(DTB[:], dtb_d.ap())
            NAE = cpool.tile([16, 1], f32, tag="nae")
            nc.sync.dma_start(NAE[:], nae_d.ap())
            DCOL = cpool.tile([128, 8], f32, tag="dcol")
            nc.sync.dma_start(DCOL[:], dcol_d.ap())
            EPSC = cpool.tile([128, 1], f32, tag="epsc")
            nc.vector.memset(EPSC[:], EPS)

            # ---------------- persistent tensors ----------------
            dtldT = mid.tile([128, NCH, 32], f32, tag="dtldT")  # 0:16 dt, 16:32 ld
            TRall = mid.tile([128, NCH, 32], f32, tag="TRall")  # 0:16 S, 16:32 lndt-S
            atot = mid.tile([16, 16], f32, tag="atot")          # [head, chunk]
            atotT = mid.tile([16, 16], f32, tag="atotT")
            atotF = mid.tile([1, 256], f32, tag="atotF")
            wdin_all = mid.tile([128, NCH, 32], f32, tag="wdin_all")
            dtw_all = mid.tile([128, NCH, 16], f32, tag="dtw_all")
            atb_all = mid.tile([128, NCH, 16], f32, tag="atb_all")
            s_sb = [mid.tile([128, NH, HD], bf16, tag=f"s_sb{i}", name=f"s_sb{i}")
                    for i in range(2)]
            xbc_c = mid.tile([128, 10, L], bf16, tag="xbc_c")
            sz = mid.tile([128, 8, L], bf16, tag="sz")
            wo = mid.tile([128, 8, D_MODEL], bf16, tag="wo")
            nc.sync.dma_start(
                wo[:], w_out_d.ap().rearrange("(ko p) m -> p ko m", p=128))

            rt_dram = dpool.tile([1152, L], bf16)

            # ============ PHASE 1: in_proj + conv + dt pipeline ============
            with (
                tc.tile_pool(name="pA", bufs=1) as pA,
                tc.tile_pool(name="pW", bufs=3) as pW,
                tc.tile_pool(name="pC", bufs=2) as pC,
                tc.tile_pool(name="ps1", bufs=4, space="PSUM") as ps1,
                tc.tile_pool(name="psT", bufs=2, space="PSUM") as psT,
                tc.tile_pool(name="psSm", bufs=2, space="PSUM") as psSm,
            ):
                dtld = pA.tile([96, L], f32, tag="dtld")  # 0:16 dt, 32:48 scr, 64:80 ld
                xTs = pA.tile([128, 4, L], bf16, tag="xTs")
                xbc_pre = pA.tile([128, 10, L + 3], bf16, tag="xbc_pre")
                xTr = xT_d.ap().rearrange("(ko p) t -> p ko t", p=128)
                wir = w_in_d.ap().rearrange("(ko p) m -> p ko m", p=128)
                for k in range(4):
                    nc.sync.dma_start(xTs[:, k, :], xTr[:, k, :])
                nc.vector.memset(xbc_pre[:, :, 0:3], 0.0)

                def conv_tile(t):
                    acc = pC.tile([128, L], bf16, tag="conv_acc")
                    nc.vector.tensor_scalar_mul(
                        acc[:], xbc_pre[:, t, 0:L], CONVW[:, t, 0:1])
                    for k in range(1, 4):
                        nc.vector.scalar_tensor_tensor(
                            acc[:], xbc_pre[:, t, k:k + L],
                            CONVW[:, t, k:k + 1], acc[:],
                            op0=OP.mult, op1=OP.add,
                        )
                    nc.scalar.activation(
                        xbc_c[:, t, :], acc[:], AF.Silu, bias=CONVB[:, t:t + 1])
                    if t < 9:
                        nc.sync.dma_start(
                            rt_dram[t * 128:(t + 1) * 128, :], xbc_c[:, t, :])

                for m in [18] + list(range(8, 18)) + list(range(0, 8)):
                    mp = 128 if m < 18 else 16
                    wis = pW.tile([128, 4, 128], bf16, tag="wis")
                    for k in range(4):
                        nc.sync.dma_start(wis[:, k, 0:mp], wir[:, k, m * 128:m * 128 + mp])
                    for tb in range(4):
                        tsl = slice(tb * 512, (tb + 1) * 512)
                        ps = ps1.tile([128, 512], f32, tag="ps_inproj")
                        for k in range(4):
                            nc.tensor.matmul(
                                ps[:mp, :], wis[:, k, 0:mp], xTs[:, k, tsl],
                                start=(k == 0), stop=(k == 3),
                            )
                        if m < 8:
                            nc.scalar.activation(sz[:, m, tsl], ps[:, :], AF.Silu)
                        elif m < 18:
                            nc.scalar.copy(
                                xbc_pre[:, m - 8, 3 + tb * 512: 3 + (tb + 1) * 512],
                                ps[:, :])
                        else:
                            nc.scalar.copy(dtld[32:48, tsl], ps[:16, :])

                    if m == 18:
                        # dt = softplus(pre + dtb); ld = -exp(A_log) * dt
                        nc.scalar.activation(dtld[32:48, :], dtld[32:48, :], AF.Exp,
                                             bias=DTB[:, 0:1])
                        nc.scalar.activation(dtld[0:16, :], dtld[32:48, :], AF.Ln,
                                             bias=1.0)
                        nc.vector.tensor_scalar_mul(
                            dtld[64:80, :], dtld[0:16, :], NAE[:, 0:1])

                        # atot per chunk = exp(chunk-sums of ld)
                        red = psSm.tile([128, 32], f32, tag="sm", name="red")
                        nc.vector.tensor_reduce(
                            red[0:16, 0:16],
                            dtld[64:80, :].rearrange("p (c t) -> p c t", c=NCH),
                            op=OP.add, axis=mybir.AxisListType.X,
                        )
                        nc.scalar.activation(atot[:], red[0:16, 0:16], AF.Exp)
                        atT_ps = psSm.tile([128, 32], f32, tag="sm", name="atT_ps")
                        nc.tensor.transpose(
                            atT_ps[0:16, 0:16], atot[:], IDNF[0:16, 0:16])
                        nc.vector.tensor_copy(atotT[:], atT_ps[0:16, 0:16])
                        nc.sync.dma_start(
                            atotF[:].rearrange("p (c h) -> p c h", c=16), atotT[:])

                        # time-major dt/ld per chunk
                        for c in range(NCH):
                            trp = psT.tile([128, 96], f32, tag="trp", name="trp")
                            nc.tensor.transpose(
                                trp[:], dtld[:, c * T:(c + 1) * T], IDNF[0:96, 0:96])
                            nc.vector.tensor_copy(dtldT[:, c, 0:16], trp[:, 0:16])
                            nc.vector.tensor_copy(dtldT[:, c, 16:32], trp[:, 64:80])

                        # decay prep A: wdin/dtw/atb/S for all chunks
                        for c in range(NCH):
                            ld_bf = pW.tile([128, 16], bf16, tag="ld_bf")
                            nc.vector.tensor_copy(ld_bf[:], dtldT[:, c, 16:32])
                            wd_ps = psSm.tile([128, 48], f32, tag="sm", name="wd_ps")
                            nc.tensor.matmul(wd_ps[:, 0:16], ALOW[:], ld_bf[:],
                                             start=True, stop=True)
                            nc.tensor.matmul(wd_ps[:, 16:32], UINC[:], ld_bf[:],
                                             start=True, stop=True)
                            nc.scalar.activation(wdin_all[:, c, :], wd_ps[:, 0:32],
                                                 AF.Exp)
                            nc.vector.tensor_copy(TRall[:, c, 0:16], wd_ps[:, 16:32])
                            nc.vector.tensor_tensor(
                                dtw_all[:, c, :], dtldT[:, c, 0:16],
                                wdin_all[:, c, 0:16], op=OP.mult)
                            if c > 0:
                                nc.tensor.matmul(
                                    wd_ps[:, 32:48], ONESRF[:],
                                    atotF[0:1, c * 16:(c + 1) * 16],
                                    start=True, stop=True)
                                nc.vector.tensor_copy(atb_all[:, c, :],
                                                      wd_ps[:, 32:48])

                    if 8 <= m < 18:
                        conv_tile(m - 8)

            if dbg:
                nc.sync.dma_start(xbcc_o.ap(), xbc_c[:])
                nc.sync.dma_start(sz_o.ap(), sz[:])

            # ============ PHASE 2: chunked scan + gating + out_proj ============
            with (
                tc.tile_pool(name="pS", bufs=2) as pS,
                tc.tile_pool(name="pXbt", bufs=3) as pXbt,
                tc.tile_pool(name="pPrep", bufs=3) as pPrep,
                tc.tile_pool(name="pGn", bufs=2) as pGn,
                tc.tile_pool(name="pYT", bufs=2) as pYT,
                tc.tile_pool(name="psE", bufs=1, space="PSUM") as psE,
                tc.tile_pool(name="psY", bufs=1, space="PSUM") as psY,
                tc.tile_pool(name="psYI", bufs=1, space="PSUM") as psYI,
                tc.tile_pool(name="psS", bufs=1, space="PSUM") as psS,
                tc.tile_pool(name="psR", bufs=1, space="PSUM") as psR,
                tc.tile_pool(name="psO", bufs=1, space="PSUM") as psO,
            ):
                PS0 = {}
                PB4 = {}
                xbt = {}
                gn_tiles = []
                yTr = yT_d.ap().rearrange("(mo p) t -> p mo t", p=128)

                def out_group(tb, dm):
                    gtile = gn_tiles[tb]
                    po = psO.tile([128, 512], f32, tag="po",
                                  name=f"po{tb}_{dm}")
                    for k in range(8):
                        nc.tensor.matmul(
                            po[:], wo[:, k, dm * 128:(dm + 1) * 128],
                            gtile[:, k, :], start=(k == 0), stop=(k == 7))
                    yv = pYT.tile([128, 512], f32, tag="yT_sb")
                    nc.scalar.copy(yv[:], po[:])
                    nc.sync.dma_start(
                        yTr[:, dm, tb * 512:(tb + 1) * 512], yv[:])

                def prep(c, sm):
                    # build P-matmul operands: S row-flat + B rows
                    lnt = pPrep.tile([128, 16], f32, tag="lnt")
                    nc.scalar.activation(lnt[:], dtldT[:, c, 0:16], AF.Ln,
                                         bias=0.0)
                    nc.vector.tensor_tensor(
                        TRall[:, c, 16:32], lnt[:], TRall[:, c, 0:16],
                        op=OP.subtract)
                    trt_ps = sm[0:32, 128:256]
                    nc.tensor.transpose(trt_ps, TRall[:, c, :], IDNF[:, :])
                    TRT = pPrep.tile([32, 128], f32, tag="TRT")
                    nc.scalar.copy(TRT[:], trt_ps)
                    # split into bf16 hi+lo so the P matmuls run in bf16 mode
                    TRTh = pPrep.tile([32, 128], bf16, tag="TRTh")
                    nc.vector.tensor_copy(TRTh[:], TRT[:])
                    TRTl = pPrep.tile([32, 128], bf16, tag="TRTl")
                    nc.vector.tensor_tensor(TRTl[:], TRT[:], TRTh[:],
                                            op=OP.subtract)
                    ps0 = pPrep.tile([2, 2048], bf16, tag="PS0")
                    nc.sync.dma_start(
                        ps0[0:1, :].rearrange("p (b i) -> p b i", b=16),
                        TRTh[0:16, :])
                    nc.sync.dma_start(
                        ps0[1:2, :].rearrange("p (b i) -> p b i", b=16),
                        TRTl[0:16, :])
                    pb4 = pPrep.tile([8, 4, 128], bf16, tag="PB4")
                    for blk in range(4):
                        nc.sync.dma_start(pb4[0:4, blk, :],
                                          TRTh[16 + 4 * blk: 20 + 4 * blk, :])
                        nc.sync.dma_start(pb4[4:8, blk, :],
                                          TRTl[16 + 4 * blk: 20 + 4 * blk, :])
                    PS0[c] = ps0
                    PB4[c] = pb4

                def xbt_load(c):
                    csl = slice(c * T, (c + 1) * T)
                    xb = pXbt.tile([128, 1152], bf16, tag="xbt")
                    nc.sync.dma_start_transpose(xb[:, 0:1024], rt_dram[0:1024, csl])
                    nc.sync.dma_start_transpose(xb[:, 1024:1152],
                                                rt_dram[1024:1152, csl])
                    xbt[c] = xb

                sm0 = psR.tile([128, 385], f32, tag="smail", name="sm0")
                prep(0, sm0)
                prep(1, sm0)
                xbt_load(0)
                xbt_load(1)

                for c in range(NCH):
                    # one single-bank PSUM tile; small users live as disjoint
                    # regions so their deps stay region-local
                    sm = psR.tile([128, 385], f32, tag="smail", name=f"sm{c}")
                    if c + 2 < NCH:
                        prep(c + 2, sm)
                        xbt_load(c + 2)
                    csl = slice(c * T, (c + 1) * T)
                    xb = xbt.pop(c)
                    ps0 = PS0.pop(c)
                    pb4 = PB4.pop(c)

                    # gt = B^T C (channel-major operands)
                    gt_ps = sm[:, 0:128]
                    nc.tensor.matmul(gt_ps, xbc_c[:, 8, csl], xbc_c[:, 9, csl],
                                     start=True, stop=True)
                    gt = pS.tile([128, 128], bf16, tag="gt")
                    nc.scalar.copy(gt[:], gt_ps)

                    y_ps = psY.tile([128, 8, T], f32, tag="y_ps")

                    # inter-Y first (time-major, scaled), so transposes can
                    # interleave with intra starts per region below
                    yw_half = []
                    if c > 0:
                        for hf in range(2):
                            yi_ps = psYI.tile([128, 8, HD], f32, tag="yi_ps")
                            nc.tensor.matmul(
                                yi_ps[:], xbc_c[:, 9, csl],
                                s_sb[(c + 1) % 2][:, hf * 8:(hf + 1) * 8, :],
                                start=True, stop=True)
                            yw = pS.tile([128, 512], bf16, tag="yw")
                            nc.vector.tensor_tensor(
                                yw[:].rearrange("p (h d) -> p h d", h=8), yi_ps[:],
                                wdin_all[:, c, 16 + hf * 8: 16 + (hf + 1) * 8, None]
                                .to_broadcast([128, 8, HD]),
                                op=OP.mult)
                            yw_half.append(yw)

                    for hb in range(4):
                        e_ps = psE.tile([128, 4, 128], f32, tag="e_ps")
                        nc.tensor.matmul(
                            e_ps[:], ONES2[:], ps0[0:2, hb * 512:(hb + 1) * 512],
                            start=True, stop=False)
                        nc.tensor.matmul(
                            e_ps[:], pb4[0:8, hb, :], IND8[:],
                            start=False, stop=False, skip_group_check=True)
                        nc.tensor.matmul(
                            e_ps[:], IDNB[:], MINF4[:],
                            start=False, stop=True, skip_group_check=True)
                        e_sb = pS.tile([128, 4, 128], bf16, tag="e_sb")
                        nc.scalar.activation(e_sb[:], e_ps[:], AF.Exp)
                        m_sb = pS.tile([128, 4, 128], bf16, tag="m_sb")
                        nc.vector.tensor_tensor(
                            m_sb[:], gt[:, None, :].to_broadcast([128, 4, 128]),
                            e_sb[:], op=OP.mult)
                        for hq in range(4):
                            h = hb * 4 + hq
                            ph, fh = (h % 2) * 64, h // 2
                            nc.tensor.matmul(
                                y_ps[ph:ph + 64, fh, :],
                                xb[:, h * HD:(h + 1) * HD], m_sb[:, hq, :],
                                start=True, stop=(c == 0),
                                skip_group_check=True)
                            if c > 0:
                                # accumulate this head's inter contribution
                                # before the next start=True clears the bank's
                                # has_written bits
                                nc.tensor.matmul(
                                    y_ps[ph:ph + 64, fh, :],
                                    yw_half[h // 8][:, (h % 8) * HD:
                                                    (h % 8 + 1) * HD],
                                    IDNB[:],
                                    start=False, stop=True,
                                    skip_group_check=True)

                    # state for this chunk
                    xdtw = pS.tile([128, NH, HD], bf16, tag="xdtw")
                    nc.gpsimd.tensor_tensor(
                        xdtw[:], xb[:, 0:1024].rearrange("p (h d) -> p h d", h=NH),
                        dtw_all[:, c, :, None].to_broadcast([128, NH, HD]),
                        op=OP.mult)
                    s_ps = psS.tile([128, NH, HD], f32, tag="s_ps")
                    nc.tensor.matmul(s_ps[:, 0:8, :], xb[:, 1024:1152],
                                     xdtw[:, 0:8, :], start=True, stop=True)
                    nc.tensor.matmul(s_ps[:, 8:16, :], xb[:, 1024:1152],
                                     xdtw[:, 8:16, :], start=True, stop=True)
                    if c == 0:
                        nc.vector.tensor_copy(s_sb[0][:], s_ps[:])
                    else:
                        s_scaled = pS.tile([128, NH, HD], bf16, tag="s_scaled")
                        nc.gpsimd.tensor_tensor(
                            s_scaled[:], s_sb[(c + 1) % 2][:],
                            atb_all[:, c, :, None].to_broadcast([128, NH, HD]),
                            op=OP.mult)
                        nc.vector.tensor_tensor(
                            s_sb[c % 2][:], s_scaled[:], s_ps[:], op=OP.add)

                    # y evac + gating + rmsnorm for this chunk
                    y_ch = pS.tile([128, 8, T], bf16, tag="y_ch")
                    for t in range(8):
                        nc.vector.scalar_tensor_tensor(
                            y_ch[:, t, :], xbc_c[:, t, csl], DCOL[:, t:t + 1],
                            y_ps[:, t, :], op0=OP.mult, op1=OP.add)
                    g_ch = pS.tile([128, 8, T], bf16, tag="g_ch")
                    nc.vector.tensor_tensor(g_ch[:], y_ch[:], sz[:, :, csl],
                                            op=OP.mult)
                    g2 = pS.tile([128, 8, T], bf16, tag="g2")
                    nc.scalar.square(g2[:], g_ch[:])
                    ss_ps = sm[:, 384:385]
                    for t in range(8):
                        nc.tensor.matmul(ss_ps, g2[:, t, :], ONEC[:],
                                         start=(t == 0), stop=(t == 7))
                    lnv = pPrep.tile([128, 1], f32, tag="lnv")
                    nc.scalar.activation(lnv[:], ss_ps, AF.Ln,
                                         bias=EPSC[:, 0:1], scale=1.0 / D_INNER)
                    rstd = pPrep.tile([128, 1], f32, tag="rstd")
                    nc.scalar.activation(rstd[:], lnv[:], AF.Exp, scale=-0.5)
                    rsT_ps = sm[0:1, 256:384]
                    nc.tensor.transpose(rsT_ps, rstd[:], IDNF[:, :])
                    rsT = pPrep.tile([1, 128], bf16, tag="rsT")
                    nc.scalar.copy(rsT[:], rsT_ps)
                    rb_ps = sm[:, 0:128]
                    nc.tensor.matmul(rb_ps, ONESRB[:], rsT[:],
                                     start=True, stop=True)
                    rb_sb = pS.tile([128, 128], bf16, tag="rb_sb")
                    nc.scalar.copy(rb_sb[:], rb_ps)

                    if c % 4 == 0:
                        gn_tb = pGn.tile([128, 8, 512], bf16, tag="gn_tb",
                                         name=f"gn_tb{c // 4}")
                        gn_tiles.append(gn_tb)
                    nc.vector.tensor_tensor(
                        gn_tb[:, :, (c % 4) * T:(c % 4 + 1) * T], g_ch[:],
                        rb_sb[:, None, :].to_broadcast([128, 8, 128]),
                        op=OP.mult)

                    if dbg:
                        nc.sync.dma_start(y_o.ap()[:, :, csl], y_ch[:])
                        nc.sync.dma_start(
                            gn_o.ap()[:, :, csl],
                            gn_tb[:, :, (c % 4) * T:(c % 4 + 1) * T])
                        nc.sync.dma_start(
                            s_o.ap()[:, c, :],
                            s_sb[c % 2][:].rearrange("p a b -> p (a b)"))

                    # out_proj: one dm-group per chunk, round-robin over
                    # finished 512-column blocks
                    if c >= 3:
                        out_group((c - 3) // 4, (c - 3) % 4)

                for i in range(13, 16):
                    out_group(3, i - 12)

    _fix_waits(nc, mybir)

    return nc


def _fix_waits(nc, mybir):
    """This walrus build supports one sem-wait slot per instruction; hoist
    excess waits onto preceding NoOps on the same engine."""
    nwn = [0]
    for bb in nc.main_func.blocks:
        newl = []
        changed = False
        for inst in bb.instructions:
            si = inst.sync_info
            waits = list(si.on_wait) if (si and si.on_wait) else []
            if len(waits) > 1:
                imm = [w for w in waits if w.wait_reg is None]
                reg = [w for w in waits if w.wait_reg is not None]
                keep = (reg + imm)[:1]
                spill = [w for w in waits if w not in keep]
                assert not any(w.wait_reg is not None for w in spill), inst.name
                for w in spill:
                    nwn[0] += 1
                    nop = mybir.InstNoOp(name=f"I-wsplit-{nwn[0]}", ins=[], outs=[])
                    nop.engine = inst.engine
                    nop.sync_info = mybir.SyncInfo(on_wait=[w], on_update=[])
                    nc.register_instruction(nop)
                    newl.append(nop)
                si.on_wait = keep
                changed = True
            newl.append(inst)
        if changed:
            bb.instructions = newl
    return nc


def _get_program(dbg=False):
    key = "nc_dbg" if dbg else "nc"
    if key not in _CACHE:
        _CACHE[key] = _build_program(dbg=dbg)
    return _CACHE[key]


def _host_consts():
    if "consts" in _CACHE:
        return _CACHE["consts"]
    import ml_dtypes
    k = np.arange(128)
    alow = (k[:, None] > k[None, :]).astype(np.float32)      # [k > j]
    uinc = (k[:, None] <= k[None, :]).astype(np.float32)     # [k <= i]
    idn = np.eye(128, dtype=np.float32)
    ind4 = np.repeat((np.arange(4)[:, None] == np.arange(4)[None, :])
                     .astype(np.float32)[:, :, None], 128, axis=2).reshape(4, 512)
    consts = dict(
        ind8=np.tile(ind4, (2, 1)).astype(ml_dtypes.bfloat16),
        ones2=np.ones((2, 128), ml_dtypes.bfloat16),
        alow=alow.astype(ml_dtypes.bfloat16),
        uinc=uinc.astype(ml_dtypes.bfloat16),
        idnb=idn.astype(ml_dtypes.bfloat16),
        idnf=idn,
        ones=np.ones((128, 1), ml_dtypes.bfloat16),
        onesrf=np.ones((1, 128), np.float32),
        onesrb=np.ones((1, 128), ml_dtypes.bfloat16),
        minf4=np.tile((k[:, None] > k[None, :]).astype(np.float32) * NEG_INF,
                      (1, 4)).astype(ml_dtypes.bfloat16),
    )
    _CACHE["consts"] = consts
    return consts


def _core_inputs(x_seq, p):
    """x_seq: (L, D_MODEL) f32 (already flipped for bw); p: dict of params."""
    import ml_dtypes
    consts = _host_consts()
    dcol = p["D"].astype(np.float32).repeat(HD).reshape(8, 128).T.copy()
    convw = np.ascontiguousarray(
        p["conv_w"].astype(np.float32).reshape(4, 10, 128).transpose(2, 1, 0)
    )
    convb = np.ascontiguousarray(p["conv_b"].astype(np.float32).reshape(10, 128).T)
    w_out = (p["norm_w"].astype(np.float32)[:, None]
             * p["out_proj"].astype(np.float32))
    return dict(
        xT=np.ascontiguousarray(x_seq.T).astype(ml_dtypes.bfloat16),
        w_in=np.ascontiguousarray(p["in_proj"]).astype(ml_dtypes.bfloat16),
        w_out=np.ascontiguousarray(w_out).astype(ml_dtypes.bfloat16),
        convw=convw,
        convb=convb,
        dtb=p["dt_bias"].astype(np.float32).reshape(16, 1),
        nae=(-np.exp(p["A_log"].astype(np.float32))).reshape(16, 1),
        dcol=dcol,
        **consts,
    )


def kernel(**inputs):
    from concourse.bass_utils import run_bass_kernel_spmd

    nc = _get_program()
    x = np.asarray(inputs["x"], np.float32)
    mask = np.asarray(inputs["padding_mask"])

    def params(pre):
        names = ["in_proj", "conv_w", "conv_b", "dt_bias", "A_log", "D", "norm_w", "out_proj"]
        return {n: np.asarray(inputs[pre + n]) for n in names}

    pf, pb = params("fw_"), params("bw_")
    in_maps = []
    for b in range(B_SZ):
        in_maps.append(_core_inputs(x[b], pf))
    for b in range(B_SZ):
        in_maps.append(_core_inputs(x[b][::-1], pb))

    res = run_bass_kernel_spmd(nc, in_maps, core_ids=list(range(8)))
    out = np.empty((B_SZ, L, D_MODEL), np.float32)
    for b in range(B_SZ):
        yf = res.results[b]["yT"].T
        yb = res.results[B_SZ + b]["yT"].T[::-1]
        out[b] = yf + yb
    out[mask] = 0.0
    return out
